# revision 45
# baseline (speedup 1.0000x reference)
"""Trainium2 Bass kernel for nn_CeptaContextBlock (B=4, T=4096, D=1024, P=512, ALPHA=4, PR=64).

Math (after algebraic simplification of the reference):
    W_comb = W_toP + sum_a W_U[:,:,a] * W_V[:,a]          (host precompute)
    WB     = W_comb @ B_mat                               (host precompute)
    Fg   = sigmoid(x @ W_F)                               (B,T,P)
    lam  = sigmoid(Fg @ W_lam)                            (B,T,PR)
    u    = x @ WB          (== (x @ W_comb) @ B_mat)      (B,T,PR)
    s    = scan: s_i = lam_i * s_{i-1} + u_i along T      (B,T,PR)
    t_til= x @ W_comb + s @ C_mat                         (B,T,P)
    h    = t_til @ W_fromP                                (B,T,D)

Sharding: 8 cores; core c handles batch b=c//2, token half c%2 (2048 tokens).
The cross-half scan dependency is NOT exchanged on device (an AllGather costs
~34us doorbell-to-data). Instead each core outputs its final scan state
sfin=s[TL-1] (64 floats) and cp[r,t]=prod_{i<=t} lam[r,i] over its first CT
tokens; cp underflows to exactly 0 within ~200 tokens. The host applies the
bilinear correction h_odd[0:CT] += (cp_odd * sfin_even[:,None]).T @ (C@W_fromP)
during the gather step (f32, ~0.03% of the model FLOPs).

On-device schedule: chunk-sequential pipeline (A_c, T_c, H_c) x 4:
  A_c = Fg (chunk 0: k-outer 4-wide to match the DMA ramp; else k-inner),
        u, lam, chained scan
  T_c = 9-matmul PSUM groups (8x wcomb + 1x C@sloc, the close deferred one
        group so the scan/cast chain never stalls the PE) -> t_til (bf16)
  H_c = t_til @ W_fromP, per-dc copy (scalar/vector) + per-dc DMA
Every stationary is a full 128-column load (u/lam/C operands are padded on
the host; narrow LDWEIGHTS measured +95ns each). Input DMAs ride both HWDGE
queues as column slices sized to the DGE's slow first ~10us, with wcrit's
back half on the SWDGE (gpsimd) queue as a third lane. There are no
collectives and no cross-core traffic.
"""

import os
import sys

import numpy as np

for _p in ("/opt/trn_rl_repo", "/root/.axon_site/_ro/trn_rl_repo"):
    if os.path.isdir(_p) and _p not in sys.path:
        sys.path.append(_p)

import ml_dtypes

import concourse.bass as bass
import concourse.bacc as bacc
import concourse.mybir as mybir
import concourse.tile as tile
from concourse import bass_utils

B, T, D, P, ALPHA, PR = 4, 4096, 1024, 512, 4, 64
NCORES = 8
TL = T // 2          # tokens per core
KD = D // 128        # 8 d-chunks (contraction for the big matmuls)
PT = P // 128        # 4 p-tiles
CH = 512             # token chunk (free dim per matmul)
NCH = TL // CH       # 4 token chunks per core
CRIT = P + 128       # packed scan-critical weight: [W_F | WB | 0-pad] per k-chunk
CT = 256             # correction window (cumprod(lam) ~ 0 well before this)
F32 = mybir.dt.float32
BF16 = mybir.dt.bfloat16
SIG = mybir.ActivationFunctionType.Sigmoid
CPY = mybir.ActivationFunctionType.Copy
MUL = mybir.AluOpType.mult
ADD = mybir.AluOpType.add
BYP = mybir.AluOpType.bypass

_CACHE = {}


def build_program(ncores: int = NCORES):
    """Build the SPMD Tile program (same NEFF on all cores, no collectives)."""
    nc = bacc.Bacc(
        "TRN2", target_bir_lowering=False, debug=False, num_devices=ncores
    )

    # k-block-major DRAM layouts: every per-k DMA slice is one fully
    # contiguous run ([128, cols] block with adjacent partitions). The
    # narrow (64-wide) stationaries are zero-padded to 128 so every
    # LDWEIGHTS is a standard full-width load that pipelines behind the
    # in-flight matmul (narrow LDW measured +95ns serialization each).
    xt_d = nc.dram_tensor("xt", [128, NCH * KD * CH], BF16, kind="ExternalInput")
    CRD = P + PR  # wcrit DRAM stores [W_F_k | WB_k] unpadded; the SBUF pad
    # columns stay uninitialized — they only feed pu rows 64:128, never read
    wcrit_d = nc.dram_tensor("wcrit", [128, KD * CRD], BF16, kind="ExternalInput")
    wcomb_d = nc.dram_tensor("wcomb", [128, KD * P], BF16, kind="ExternalInput")
    wlam_d = nc.dram_tensor("wlam", [128, PT * 128], BF16, kind="ExternalInput")
    cmat_d = nc.dram_tensor("cmat", [128, P], BF16, kind="ExternalInput")
    wfp_d = nc.dram_tensor("wfp", [128, PT * D], BF16, kind="ExternalInput")
    h_d = nc.dram_tensor("h", [TL, D], BF16, kind="ExternalOutput")
    sfin_d = nc.dram_tensor("sfin", [PR, 1], F32, kind="ExternalOutput")
    cp_d = nc.dram_tensor("cp", [PR, CT], F32, kind="ExternalOutput")

    xt_vc = xt_d.rearrange("p (c q) -> p c q", c=NCH)

    with tile.TileContext(nc) as tc:
        with (
            tc.tile_pool(name="wp", bufs=1) as wp,
            tc.tile_pool(name="xp", bufs=4) as xp,
            tc.tile_pool(name="fgp", bufs=2) as fgp,
            tc.tile_pool(name="ttp", bufs=2) as ttp,
            tc.tile_pool(name="sp", bufs=2) as sp,
            tc.tile_pool(name="big", bufs=1) as big,
            tc.tile_pool(name="hp", bufs=8) as hp,
            tc.tile_pool(name="ppa", bufs=2, space="PSUM") as ppa,
            tc.tile_pool(name="pps", bufs=1, space="PSUM") as pps,
            tc.tile_pool(name="ppt", bufs=3, space="PSUM") as ppt,
            tc.tile_pool(name="pph", bufs=2, space="PSUM") as pph,
        ):
            # ---- input DMAs. Two HWDGE queues (sync/scalar) carry the
            # critical stream as quarter-column slices (2.3-4.6KB/partition
            # runs stream at full rate; per-descriptor latency ~1us, so 4
            # slices/tensor balances start latency vs throughput). gpsimd
            # (SWDGE, ~60GB/s) hauls the late-deadline weights as bonus
            # bandwidth. ----
            wcrit_sb = wp.tile([128, KD * CRIT], BF16, tag="wcrit", name="wcrit_sb")
            xt_tiles = [
                xp.tile([128, KD * CH], BF16, tag="xt", name=f"xt{c}")
                for c in range(NCH)
            ]
            wcv = wcrit_sb[:].rearrange("p (k q) -> p k q", k=KD)
            wdv = wcrit_d.rearrange("p (k q) -> p k q", k=KD)
            QX = 2 * CH    # xt quarter
            # sync: wcrit k0-3 then phase-T/H weights; scalar: xt0 then the
            # later x chunks; gpsimd (SWDGE ~64GB/s) is a third lane hauling
            # wcrit k4-7 + cmat in parallel with the HWDGE ramp
            # ramp slices sized to the DGE's slow first ~10us (~60GB/s/queue)
            # and interleaved across the queues so the k0 pair, then the k1
            # pair, arrive in parallel; wcrit k4-7 rides SWDGE
            def wc_dma(eng, ka, kb):
                eng.dma_start(wcv[:, ka:kb, 0:CRD], wdv[:, ka:kb, :])

            def x0_dma(eng, ka, kb):
                eng.dma_start(
                    xt_tiles[0][:, ka * CH : kb * CH],
                    xt_vc[:, 0, ka * CH : kb * CH],
                )

            wc_dma(nc.sync, 0, 1)
            x0_dma(nc.scalar, 0, 1)
            x0_dma(nc.sync, 1, 2)
            wc_dma(nc.scalar, 1, 2)
            wc_dma(nc.sync, 2, 4)
            x0_dma(nc.scalar, 2, 4)
            wc_dma(nc.gpsimd, 4, 6)
            wc_dma(nc.gpsimd, 6, 8)
            x0_dma(nc.scalar, 4, 8)
            wlam_sb = wp.tile([128, PT * 128], BF16, tag="wlam", name="wlam_sb")
            nc.sync.dma_start(wlam_sb[:], wlam_d[:, :])
            wcomb_sb = wp.tile([128, KD * P], BF16, tag="wcomb", name="wcomb_sb")
            nc.sync.dma_start(wcomb_sb[:], wcomb_d[:, :])
            wfp_sb = wp.tile([128, PT * D], BF16, tag="wfp", name="wfp_sb")
            nc.sync.dma_start(wfp_sb[:], wfp_d[:, :])
            hx = KD // 2 * CH  # xt1 split across both queues
            nc.scalar.dma_start(xt_tiles[1][:, 0:hx], xt_vc[:, 1, 0:hx])
            nc.sync.dma_start(xt_tiles[1][:, hx:], xt_vc[:, 1, hx:])
            nc.scalar.dma_start(xt_tiles[2][:], xt_vc[:, 2, :])
            nc.scalar.dma_start(xt_tiles[3][:], xt_vc[:, 3, :])
            cmat_sb = wp.tile([128, P], BF16, tag="cmat", name="cmat_sb")
            nc.gpsimd.dma_start(cmat_sb[:], cmat_d[:, :])

            # (a PE pre-warm with junk matmuls was tried and measured
            # net-neutral: the ramp is DMA-bound, so the cold-clock matmuls
            # already hide inside DMA stalls)

            # ---- persistent activations ----
            s1_sb = big.tile([PR, TL], F32, tag="s1", name="s1")
            cp_sb = big.tile([PR, CT], F32, tag="cp", name="cp")
            # sloc holds bf16 s per chunk on partitions 0:64; partitions
            # 64:128 feed the zero rows of the padded C stationary and are
            # zeroed once (uninitialized SBUF could hold NaN, and NaN*0=NaN)
            sloc_sb = big.tile([128, TL], BF16, tag="sloc", name="sloc")
            nc.vector.memset(sloc_sb[64:128, :], 0.0)

            def phase_a(c):
                """Fg, u, lam, chained scan for token chunk c."""
                cs = slice(c * CH, (c + 1) * CH)
                xt_c = xt_tiles[c]
                fg_c = [
                    fgp.tile([128, CH], BF16, tag=f"fg{m}", name=f"fg{c}_{m}")
                    for m in range(PT)
                ]
                pa = [
                    ppa.tile([128, CH], F32, tag="pa", name=f"pa{c}_{j}")
                    for j in range(2)
                ]
                if c == 0:
                    # k-outer, 4-wide: all four p-tiles advance per k-step, so
                    # the PE's data-demand rate (~150GB/s cold) matches the
                    # early DMA supply. Banks 2/3 borrow the T-pool's slots
                    # (lifetimes are sequential: sigmoids free them before T0)
                    pa = pa + [
                        ppt.tile([128, CH], F32, tag="pt", name=f"pa0x{j}")
                        for j in range(2)
                    ]
                    for k in range(KD):
                        for m in range(PT):
                            nc.tensor.matmul(
                                pa[m][:],
                                wcrit_sb[
                                    :, k * CRIT + m * 128 : k * CRIT + (m + 1) * 128
                                ],
                                xt_c[:, k * CH : (k + 1) * CH],
                                start=(k == 0),
                                stop=(k == KD - 1),
                            )
                    for m in range(PT):
                        nc.scalar.activation(fg_c[m][:], pa[m][:], SIG)
                else:
                    # k-inner, m-outer: each group's bank frees via its
                    # sigmoid while the next group runs (no convoy)
                    for m in range(PT):
                        pam = pa[m % 2] if m < 2 else ppa.tile(
                            [128, CH], F32, tag="pa", name=f"pa{c}b{m}"
                        )
                        for k in range(KD):
                            nc.tensor.matmul(
                                pam[:],
                                wcrit_sb[
                                    :, k * CRIT + m * 128 : k * CRIT + (m + 1) * 128
                                ],
                                xt_c[:, k * CH : (k + 1) * CH],
                                start=(k == 0),
                                stop=(k == KD - 1),
                            )
                        nc.scalar.activation(fg_c[m][:], pam[:], SIG)
                # u = x @ [WB | junk-pad]; pad only feeds pu rows 64:128,
                # which nothing reads
                pu = pps.tile([128, CH], F32, tag="ps", name=f"pu{c}")
                for k in range(KD):
                    nc.tensor.matmul(
                        pu[:],
                        wcrit_sb[:, k * CRIT + P : (k + 1) * CRIT],
                        xt_c[:, k * CH : (k + 1) * CH],
                        start=(k == 0),
                        stop=(k == KD - 1),
                    )
                # lam = sigmoid(Fg @ [W_lam | 0]); pl borrows a ppa slot (the
                # fg banks are sigmoided-free by the time lam runs)
                pl = ppa.tile([128, CH], F32, tag="pa", name=f"pl{c}")
                for m in range(PT):
                    nc.tensor.matmul(
                        pl[:],
                        wlam_sb[:, m * 128 : (m + 1) * 128],
                        fg_c[m][:],
                        start=(m == 0),
                        stop=(m == PT - 1),
                    )
                lam_c = sp.tile([PR, CH], F32, tag="lam", name=f"lam{c}")
                nc.scalar.activation(lam_c[:], pl[0:PR, :], SIG)
                # chained local scan; u consumed straight from PSUM
                init = 0.0 if c == 0 else s1_sb[:, c * CH - 1 : c * CH]
                nc.vector.tensor_tensor_scan(
                    s1_sb[:, cs], lam_c[:], pu[0:PR, :], init, op0=MUL, op1=ADD
                )
                if c == 0:
                    nc.vector.tensor_tensor_scan(
                        cp_sb[:], lam_c[:, 0:CT], lam_c[:, 0:CT], 1.0,
                        op0=MUL, op1=BYP,
                    )
                    nc.gpsimd.dma_start(cp_d[:, :], cp_sb[:])
                nc.vector.tensor_copy(sloc_sb[0:PR, cs], s1_sb[:, cs])
                if c == NCH - 1:
                    nc.gpsimd.dma_start(sfin_d[:, :], s1_sb[:, TL - 1 : TL])

            def phase_t(c):
                """t_til = x@W_comb + s@C as one 9-mm PSUM group per p-tile.
                The C stationary is row-padded to 128 (rows 64:128 zero), so
                the s matmul is a standard full-width load too. Each group's
                closing s@C matmul is deferred behind the NEXT group's wcomb
                run, giving the scan->sloc-cast chain ~3.5us of PE cover."""
                cs = slice(c * CH, (c + 1) * CH)
                xt_c = xt_tiles[c]
                ttil_c = [
                    ttp.tile([128, CH], BF16, tag=f"tt{m}", name=f"ttil{c}_{m}")
                    for m in range(PT)
                ]
                pt_ = [None] * PT

                def sc_close(m):
                    nc.tensor.matmul(
                        pt_[m][:],
                        cmat_sb[:, m * 128 : (m + 1) * 128],
                        sloc_sb[:, cs],
                        start=False,
                        stop=True,
                    )
                    nc.vector.tensor_copy(ttil_c[m][:], pt_[m][:])

                for m in range(PT):
                    pt_[m] = ppt.tile([128, CH], F32, tag="pt", name=f"pt{c}_{m}")
                    for k in range(KD):
                        nc.tensor.matmul(
                            pt_[m][:],
                            wcomb_sb[:, k * P + m * 128 : k * P + (m + 1) * 128],
                            xt_c[:, k * CH : (k + 1) * CH],
                            start=(k == 0),
                            stop=False,
                        )
                    if m >= 1:
                        sc_close(m - 1)
                sc_close(PT - 1)
                return ttil_c

            def phase_h(c, ttil_c):
                """h = t_til @ W_fromP, streamed out per 128-token tile."""
                for tt in range(CH // 128):
                    ts_ = slice(tt * 128, (tt + 1) * 128)
                    rs = slice((c * 4 + tt) * 128, (c * 4 + tt + 1) * 128)
                    h_t = hp.tile([128, D], BF16, tag="hs", name=f"h{c}_{tt}")
                    for dc in range(2):
                        dcs = slice(dc * CH, (dc + 1) * CH)
                        ph = pph.tile([128, CH], F32, tag="ph", name=f"ph{c}_{tt}_{dc}")
                        for m in range(PT):
                            nc.tensor.matmul(
                                ph[:],
                                ttil_c[m][:, ts_],
                                wfp_sb[:, m * D + dc * CH : m * D + dc * CH + CH],
                                start=(m == 0),
                                stop=(m == PT - 1),
                            )
                        last = c == NCH - 1 and tt == 3 and dc == 1
                        if last:
                            # final copy split across both engines; its two
                            # DMA halves ride both queues — shortest tail
                            nc.scalar.activation(
                                h_t[:, CH : CH + 256], ph[:, 0:256], CPY
                            )
                            nc.vector.tensor_copy(
                                h_t[:, CH + 256 : D], ph[:, 256:CH]
                            )
                            nc.sync.dma_start(
                                h_d[rs, CH : CH + 256], h_t[:, CH : CH + 256]
                            )
                            nc.scalar.dma_start(
                                h_d[rs, CH + 256 : D], h_t[:, CH + 256 : D]
                            )
                        else:
                            if dc == 0:
                                nc.scalar.activation(h_t[:, dcs], ph[:], CPY)
                            else:
                                nc.vector.tensor_copy(h_t[:, dcs], ph[:])
                            # per-dc DMA right behind its copy: fine-grained
                            # waits keep the output stream flowing (a per-tile
                            # DMA gets a coarsened vector-clock wait and
                            # bunches up)
                            nc.sync.dma_start(h_d[rs, dcs], h_t[:, dcs])

            # ---- chunk-sequential pipeline: chunk 0's full A/T/H covers
            # the input-DMA window, pushing xt1's deadline to ~35us ----
            for c in range(NCH):
                phase_a(c)
                tt_c = phase_t(c)
                phase_h(c, tt_c)

    nc.compile()
    return nc


def _prep_inputs(x, W_toP, W_U, W_F, W_V, W_lam, B_mat, C_mat, W_fromP):
    """Host-side sharding prep: weight folds, bf16 cast, per-core x transpose."""
    bf = ml_dtypes.bfloat16
    def swz(w):
        # [K*128, q] -> partition-major [128, K*q]
        kq = w.shape[0] // 128
        return np.ascontiguousarray(
            w.reshape(kq, 128, w.shape[1]).transpose(1, 0, 2).reshape(128, -1)
        )

    W_comb = (W_toP + (W_U * W_V[None, :, :]).sum(-1)).astype(np.float32)
    WB = W_comb @ np.asarray(B_mat, np.float32)
    # partition-major [128, KD*(P+PR)]: per k-chunk [W_F_k | WB_k], no pad
    wf32 = np.asarray(W_F, np.float32).reshape(KD, 128, P)
    wbb = WB.reshape(KD, 128, PR)
    wcrit = np.ascontiguousarray(
        np.concatenate([wf32, wbb], axis=2).transpose(1, 0, 2).reshape(128, -1)
    ).astype(bf)
    wcomb = swz(W_comb).astype(bf)
    wlam_pad = np.zeros((P, 128), np.float32)
    wlam_pad[:, 0:PR] = np.asarray(W_lam, np.float32)
    wlam = swz(wlam_pad).astype(bf)
    cmat = np.zeros((128, P), np.float32)
    cmat[0:PR, :] = np.asarray(C_mat, np.float32)
    cmat = cmat.astype(bf)
    wfp = swz(np.asarray(W_fromP, np.float32)).astype(bf)  # [128, PT*D]
    in_maps = []
    for c in range(NCORES):
        b, half = c // 2, c % 2
        xT = np.asarray(x[b, half * TL : (half + 1) * TL, :], np.float32).T
        # [D, TL] -> [128, NCH*KD*CH] with (c, k, t) free order, partition-major
        xs = np.ascontiguousarray(
            xT.reshape(KD, 128, NCH, CH).transpose(1, 2, 0, 3).reshape(128, -1)
        ).astype(bf)
        in_maps.append(
            {
                "xt": xs,
                "wcrit": wcrit,
                "wcomb": wcomb,
                "wlam": wlam,
                "cmat": cmat,
                "wfp": wfp,
            }
        )
    return in_maps


def kernel(**inputs) -> np.ndarray:
    inputs = {k: np.asarray(v) for k, v in inputs.items()}
    if "nc" not in _CACHE:
        _CACHE["nc"] = build_program()
    nc = _CACHE["nc"]
    in_maps = _prep_inputs(**inputs)
    trace = bool(int(os.environ.get("CEPTA_TRACE", "0")))
    res = bass_utils.run_bass_kernel_spmd(
        nc,
        in_maps,
        core_ids=list(range(NCORES)),
        trace=trace,
        trace_cores=[0] if trace else None,
    )
    _CACHE["last_result"] = res
    # host-side gather + cross-half scan-carry correction (bilinear in the
    # tiny cp [PR,CT] and sfin [PR] outputs; f32, ~0.03% of model FLOPs)
    mcw = (
        np.asarray(inputs["C_mat"], np.float32)
        @ np.asarray(inputs["W_fromP"], np.float32)
    )
    out = np.empty((B, T, D), np.float32)
    for b in range(B):
        even, odd = res.results[2 * b], res.results[2 * b + 1]
        h0 = even["h"].astype(np.float32)
        h1 = odd["h"].astype(np.float32)
        corr = (odd["cp"] * even["sfin"]).T @ mcw   # [CT, D]
        h1[0:CT] += corr
        out[b, 0:TL] = h0
        out[b, TL:T] = h1
    return out


# revision 48
# speedup vs baseline: 1.0364x; 1.0364x over previous
"""Trainium2 Bass kernel for nn_CeptaContextBlock (B=4, T=4096, D=1024, P=512, ALPHA=4, PR=64).

Math (after algebraic simplification of the reference):
    W_comb = W_toP + sum_a W_U[:,:,a] * W_V[:,a]          (host precompute)
    WB     = W_comb @ B_mat                               (host precompute)
    Fg   = sigmoid(x @ W_F)                               (B,T,P)
    lam  = sigmoid(Fg @ W_lam)                            (B,T,PR)
    u    = x @ WB          (== (x @ W_comb) @ B_mat)      (B,T,PR)
    s    = scan: s_i = lam_i * s_{i-1} + u_i along T      (B,T,PR)
    t_til= x @ W_comb + s @ C_mat                         (B,T,P)
    h    = t_til @ W_fromP                                (B,T,D)

Sharding: 8 cores; core c handles batch b=c//2, token half c%2 (2048 tokens).
The cross-half scan dependency is NOT exchanged on device (an AllGather costs
~34us doorbell-to-data). Instead each core outputs its final scan state
sfin=s[TL-1] (64 floats) and cp[r,t]=prod_{i<=t} lam[r,i] over its first CT
tokens; cp underflows to exactly 0 within ~200 tokens. The host applies the
bilinear correction h_odd[0:CT] += (cp_odd * sfin_even[:,None]).T @ (C@W_fromP)
during the gather step (f32, ~0.03% of the model FLOPs).

On-device schedule: chunk-sequential pipeline (A_c, T_c, H_c) x 4:
  A_c = Fg (chunk 0: k-outer 4-wide to match the DMA ramp; else k-inner),
        u, lam, chained scan
  T_c = 9-matmul PSUM groups (8x wcomb + 1x C@sloc, the close deferred one
        group so the scan/cast chain never stalls the PE) -> t_til (bf16)
  H_c = t_til @ W_fromP, per-dc copy (scalar/vector) + per-dc DMA
Every stationary is a full 128-column load (u/lam/C operands are padded on
the host; narrow LDWEIGHTS measured +95ns each). Input DMAs ride both HWDGE
queues as column slices sized to the DGE's slow first ~10us, with wcrit's
back half on the SWDGE (gpsimd) queue as a third lane. There are no
collectives and no cross-core traffic.
"""

import os
import sys

import numpy as np

for _p in ("/opt/trn_rl_repo", "/root/.axon_site/_ro/trn_rl_repo"):
    if os.path.isdir(_p) and _p not in sys.path:
        sys.path.append(_p)

import ml_dtypes

import concourse.bass as bass
import concourse.bacc as bacc
import concourse.mybir as mybir
import concourse.tile as tile
from concourse import bass_utils

B, T, D, P, ALPHA, PR = 4, 4096, 1024, 512, 4, 64
NCORES = 8
TL = T // 2          # tokens per core
KD = D // 128        # 8 d-chunks (contraction for the big matmuls)
PT = P // 128        # 4 p-tiles
CH = 512             # token chunk (free dim per matmul)
NCH = TL // CH       # 4 token chunks per core
CRIT = P + 128       # packed scan-critical weight: [W_F | WB | 0-pad] per k-chunk
CT = 256             # correction window (cumprod(lam) ~ 0 well before this)
F32 = mybir.dt.float32
BF16 = mybir.dt.bfloat16
F8 = mybir.dt.float8e4
DR = mybir.MatmulPerfMode.DoubleRow
SIG = mybir.ActivationFunctionType.Sigmoid
CPY = mybir.ActivationFunctionType.Copy
MUL = mybir.AluOpType.mult
ADD = mybir.AluOpType.add
BYP = mybir.AluOpType.bypass

_CACHE = {}


def build_program(ncores: int = NCORES):
    """Build the SPMD Tile program (same NEFF on all cores, no collectives)."""
    nc = bacc.Bacc(
        "TRN2", target_bir_lowering=False, debug=False, num_devices=ncores
    )

    # k-block-major DRAM layouts: every per-k DMA slice is one fully
    # contiguous run ([128, cols] block with adjacent partitions). The
    # narrow (64-wide) stationaries are zero-padded to 128 so every
    # LDWEIGHTS is a standard full-width load that pipelines behind the
    # in-flight matmul (narrow LDW measured +95ns serialization each).
    xt_d = nc.dram_tensor("xt", [128, NCH * KD * CH], BF16, kind="ExternalInput")
    x8_d = nc.dram_tensor("x8", [128, NCH * KD * CH], F8, kind="ExternalInput")
    wf8_d = nc.dram_tensor("wf8", [128, KD * P], F8, kind="ExternalInput")
    wb_d = nc.dram_tensor("wb", [128, KD * 128], BF16, kind="ExternalInput")
    wcomb_d = nc.dram_tensor("wcomb", [128, KD * P], BF16, kind="ExternalInput")
    wlam_d = nc.dram_tensor("wlam", [128, PT * 128], BF16, kind="ExternalInput")
    cmat_d = nc.dram_tensor("cmat", [128, P], BF16, kind="ExternalInput")
    wfp_d = nc.dram_tensor("wfp", [128, PT * D], BF16, kind="ExternalInput")
    h_d = nc.dram_tensor("h", [TL, D], BF16, kind="ExternalOutput")
    sfin_d = nc.dram_tensor("sfin", [PR, 1], F32, kind="ExternalOutput")
    cp_d = nc.dram_tensor("cp", [PR, CT], F32, kind="ExternalOutput")

    xt_vc = xt_d.rearrange("p (c q) -> p c q", c=NCH)
    x8_vc = x8_d.rearrange("p (c q) -> p c q", c=NCH)

    with tile.TileContext(nc) as tc:
        with (
            tc.tile_pool(name="wp", bufs=1) as wp,
            tc.tile_pool(name="xp", bufs=4) as xp,
            tc.tile_pool(name="fgp", bufs=2) as fgp,
            tc.tile_pool(name="ttp", bufs=2) as ttp,
            tc.tile_pool(name="sp", bufs=2) as sp,
            tc.tile_pool(name="big", bufs=1) as big,
            tc.tile_pool(name="hp", bufs=8) as hp,
            tc.tile_pool(name="ppa", bufs=2, space="PSUM") as ppa,
            tc.tile_pool(name="pps", bufs=1, space="PSUM") as pps,
            tc.tile_pool(name="ppt", bufs=3, space="PSUM") as ppt,
            tc.tile_pool(name="pph", bufs=2, space="PSUM") as pph,
        ):
            # ---- input DMAs. Two HWDGE queues (sync/scalar) carry the
            # critical stream as quarter-column slices (2.3-4.6KB/partition
            # runs stream at full rate; per-descriptor latency ~1us, so 4
            # slices/tensor balances start latency vs throughput). gpsimd
            # (SWDGE, ~60GB/s) hauls the late-deadline weights as bonus
            # bandwidth. ----
            wf8_sb = wp.tile([128, KD * P], F8, tag="wf8", name="wf8_sb")
            wb_sb = wp.tile([128, KD * 128], BF16, tag="wb", name="wb_sb")
            xt_tiles = [
                xp.tile([128, KD * CH], BF16, tag="xt", name=f"xt{c}")
                for c in range(NCH)
            ]
            x8_tiles = [
                xp.tile([128, KD * CH], F8, tag="x8", name=f"x8_{c}")
                for c in range(NCH)
            ]
            # ramp (sync/scalar HWDGE): fp8 W_F + fp8 x chunk 0 in quarter
            # slices — the Fg path needs only ~1MB before full speed
            for q in range(4):
                nc.sync.dma_start(
                    wf8_sb[:, q * 2 * P : (q + 1) * 2 * P],
                    wf8_d[:, q * 2 * P : (q + 1) * 2 * P],
                )
                nc.scalar.dma_start(
                    x8_tiles[0][:, q * 2 * CH : (q + 1) * 2 * CH],
                    x8_vc[:, 0, q * 2 * CH : (q + 1) * 2 * CH],
                )
            # u/t-phase inputs behind the ramp
            nc.scalar.dma_start(xt_tiles[0][:], xt_vc[:, 0, :])
            nc.sync.dma_start(wb_sb[:], wb_d[:, :])
            nc.sync.dma_start(x8_tiles[1][:], x8_vc[:, 1, :])
            wlam_sb = wp.tile([128, PT * 128], BF16, tag="wlam", name="wlam_sb")
            nc.sync.dma_start(wlam_sb[:], wlam_d[:, :])
            wcomb_sb = wp.tile([128, KD * P], BF16, tag="wcomb", name="wcomb_sb")
            nc.sync.dma_start(wcomb_sb[:], wcomb_d[:, :])
            wfp_sb = wp.tile([128, PT * D], BF16, tag="wfp", name="wfp_sb")
            nc.sync.dma_start(wfp_sb[:], wfp_d[:, :])
            nc.scalar.dma_start(xt_tiles[1][:], xt_vc[:, 1, :])
            nc.scalar.dma_start(xt_tiles[2][:], xt_vc[:, 2, :])
            nc.sync.dma_start(x8_tiles[2][:], x8_vc[:, 2, :])
            nc.scalar.dma_start(xt_tiles[3][:], xt_vc[:, 3, :])
            nc.sync.dma_start(x8_tiles[3][:], x8_vc[:, 3, :])
            cmat_sb = wp.tile([128, P], BF16, tag="cmat", name="cmat_sb")
            nc.gpsimd.dma_start(cmat_sb[:], cmat_d[:, :])

            # (a PE pre-warm with junk matmuls was tried and measured
            # net-neutral: the ramp is DMA-bound, so the cold-clock matmuls
            # already hide inside DMA stalls)

            # ---- persistent activations ----
            s1_sb = big.tile([PR, TL], F32, tag="s1", name="s1")
            cp_sb = big.tile([PR, CT], F32, tag="cp", name="cp")
            # sloc holds bf16 s per chunk on partitions 0:64; partitions
            # 64:128 feed the zero rows of the padded C stationary and are
            # zeroed once (uninitialized SBUF could hold NaN, and NaN*0=NaN)
            sloc_sb = big.tile([128, TL], BF16, tag="sloc", name="sloc")
            nc.vector.memset(sloc_sb[64:128, :], 0.0)

            def phase_a(c):
                """Fg (fp8 DoubleRow), u, lam, chained scan for chunk c."""
                cs = slice(c * CH, (c + 1) * CH)
                xt_c = xt_tiles[c]
                # 3D views: [p, 2, f] pairs two k-chunks per DoubleRow matmul
                # (out = sum_i W[:,i].T @ X[:,i] — a 256-deep contraction)
                wfv = wf8_sb[:].rearrange("p (k q) -> p k q", k=KD)
                x8v = x8_tiles[c][:].rearrange("p (k q) -> p k q", k=KD)
                fg_c = [
                    fgp.tile([128, CH], BF16, tag=f"fg{m}", name=f"fg{c}_{m}")
                    for m in range(PT)
                ]
                pa = [
                    ppa.tile([128, CH], F32, tag="pa", name=f"pa{c}_{j}")
                    for j in range(2)
                ]
                if c == 0:
                    # pair-outer, 4-wide: all four p-tiles advance per k-pair,
                    # matching the DMA ramp. Banks 2/3 borrow the T-pool's
                    # slots (sequential lifetimes: sigmoids free them pre-T0)
                    pa = pa + [
                        ppt.tile([128, CH], F32, tag="pt", name=f"pa0x{j}")
                        for j in range(2)
                    ]
                    for q in range(KD // 2):
                        for m in range(PT):
                            nc.tensor.matmul(
                                pa[m][:],
                                wfv[:, 2 * q : 2 * q + 2, m * 128 : (m + 1) * 128],
                                x8v[:, 2 * q : 2 * q + 2, :],
                                start=(q == 0),
                                stop=(q == KD // 2 - 1),
                                perf_mode=DR,
                            )
                    for m in range(PT):
                        nc.scalar.activation(fg_c[m][:], pa[m][:], SIG)
                else:
                    # pair-inner, m-outer: each group's bank frees via its
                    # sigmoid while the next group runs (no convoy)
                    for m in range(PT):
                        pam = pa[m % 2] if m < 2 else ppa.tile(
                            [128, CH], F32, tag="pa", name=f"pa{c}b{m}"
                        )
                        for q in range(KD // 2):
                            nc.tensor.matmul(
                                pam[:],
                                wfv[:, 2 * q : 2 * q + 2, m * 128 : (m + 1) * 128],
                                x8v[:, 2 * q : 2 * q + 2, :],
                                start=(q == 0),
                                stop=(q == KD // 2 - 1),
                                perf_mode=DR,
                            )
                        nc.scalar.activation(fg_c[m][:], pam[:], SIG)
                # u = x @ [WB | 0] (bf16 x; the scan consumes u directly, so
                # it cannot ride the fp8 path)
                pu = pps.tile([128, CH], F32, tag="ps", name=f"pu{c}")
                for k in range(KD):
                    nc.tensor.matmul(
                        pu[:],
                        wb_sb[:, k * 128 : (k + 1) * 128],
                        xt_c[:, k * CH : (k + 1) * CH],
                        start=(k == 0),
                        stop=(k == KD - 1),
                    )
                # lam = sigmoid(Fg @ [W_lam | 0]); pl borrows a ppa slot (the
                # fg banks are sigmoided-free by the time lam runs)
                pl = ppa.tile([128, CH], F32, tag="pa", name=f"pl{c}")
                for m in range(PT):
                    nc.tensor.matmul(
                        pl[:],
                        wlam_sb[:, m * 128 : (m + 1) * 128],
                        fg_c[m][:],
                        start=(m == 0),
                        stop=(m == PT - 1),
                    )
                lam_c = sp.tile([PR, CH], F32, tag="lam", name=f"lam{c}")
                nc.scalar.activation(lam_c[:], pl[0:PR, :], SIG)
                # chained local scan; u consumed straight from PSUM
                init = 0.0 if c == 0 else s1_sb[:, c * CH - 1 : c * CH]
                nc.vector.tensor_tensor_scan(
                    s1_sb[:, cs], lam_c[:], pu[0:PR, :], init, op0=MUL, op1=ADD
                )
                if c == 0:
                    nc.vector.tensor_tensor_scan(
                        cp_sb[:], lam_c[:, 0:CT], lam_c[:, 0:CT], 1.0,
                        op0=MUL, op1=BYP,
                    )
                    nc.gpsimd.dma_start(cp_d[:, :], cp_sb[:])
                nc.vector.tensor_copy(sloc_sb[0:PR, cs], s1_sb[:, cs])
                if c == NCH - 1:
                    nc.gpsimd.dma_start(sfin_d[:, :], s1_sb[:, TL - 1 : TL])

            def phase_t(c):
                """t_til = x@W_comb + s@C as one 9-mm PSUM group per p-tile.
                The C stationary is row-padded to 128 (rows 64:128 zero), so
                the s matmul is a standard full-width load too. Each group's
                closing s@C matmul is deferred behind the NEXT group's wcomb
                run, giving the scan->sloc-cast chain ~3.5us of PE cover."""
                cs = slice(c * CH, (c + 1) * CH)
                xt_c = xt_tiles[c]
                ttil_c = [
                    ttp.tile([128, CH], BF16, tag=f"tt{m}", name=f"ttil{c}_{m}")
                    for m in range(PT)
                ]
                pt_ = [None] * PT

                def sc_close(m):
                    nc.tensor.matmul(
                        pt_[m][:],
                        cmat_sb[:, m * 128 : (m + 1) * 128],
                        sloc_sb[:, cs],
                        start=False,
                        stop=True,
                    )
                    nc.vector.tensor_copy(ttil_c[m][:], pt_[m][:])

                for m in range(PT):
                    pt_[m] = ppt.tile([128, CH], F32, tag="pt", name=f"pt{c}_{m}")
                    for k in range(KD):
                        nc.tensor.matmul(
                            pt_[m][:],
                            wcomb_sb[:, k * P + m * 128 : k * P + (m + 1) * 128],
                            xt_c[:, k * CH : (k + 1) * CH],
                            start=(k == 0),
                            stop=False,
                        )
                    if m >= 1:
                        sc_close(m - 1)
                sc_close(PT - 1)
                return ttil_c

            def phase_h(c, ttil_c):
                """h = t_til @ W_fromP, streamed out per 128-token tile."""
                for tt in range(CH // 128):
                    ts_ = slice(tt * 128, (tt + 1) * 128)
                    rs = slice((c * 4 + tt) * 128, (c * 4 + tt + 1) * 128)
                    h_t = hp.tile([128, D], BF16, tag="hs", name=f"h{c}_{tt}")
                    for dc in range(2):
                        dcs = slice(dc * CH, (dc + 1) * CH)
                        ph = pph.tile([128, CH], F32, tag="ph", name=f"ph{c}_{tt}_{dc}")
                        for m in range(PT):
                            nc.tensor.matmul(
                                ph[:],
                                ttil_c[m][:, ts_],
                                wfp_sb[:, m * D + dc * CH : m * D + dc * CH + CH],
                                start=(m == 0),
                                stop=(m == PT - 1),
                            )
                        last = c == NCH - 1 and tt == 3 and dc == 1
                        if last:
                            # final copy split across both engines; its two
                            # DMA halves ride both queues — shortest tail
                            nc.scalar.activation(
                                h_t[:, CH : CH + 256], ph[:, 0:256], CPY
                            )
                            nc.vector.tensor_copy(
                                h_t[:, CH + 256 : D], ph[:, 256:CH]
                            )
                            nc.sync.dma_start(
                                h_d[rs, CH : CH + 256], h_t[:, CH : CH + 256]
                            )
                            nc.scalar.dma_start(
                                h_d[rs, CH + 256 : D], h_t[:, CH + 256 : D]
                            )
                        else:
                            if dc == 0:
                                nc.scalar.activation(h_t[:, dcs], ph[:], CPY)
                            else:
                                nc.vector.tensor_copy(h_t[:, dcs], ph[:])
                            # per-dc DMA right behind its copy: fine-grained
                            # waits keep the output stream flowing (a per-tile
                            # DMA gets a coarsened vector-clock wait and
                            # bunches up)
                            nc.sync.dma_start(h_d[rs, dcs], h_t[:, dcs])

            # ---- chunk-sequential pipeline: chunk 0's full A/T/H covers
            # the input-DMA window, pushing xt1's deadline to ~35us ----
            for c in range(NCH):
                phase_a(c)
                tt_c = phase_t(c)
                phase_h(c, tt_c)

    nc.compile()
    return nc


def _prep_inputs(x, W_toP, W_U, W_F, W_V, W_lam, B_mat, C_mat, W_fromP):
    """Host-side sharding prep: weight folds, bf16 cast, per-core x transpose."""
    bf = ml_dtypes.bfloat16
    def swz(w):
        # [K*128, q] -> partition-major [128, K*q]
        kq = w.shape[0] // 128
        return np.ascontiguousarray(
            w.reshape(kq, 128, w.shape[1]).transpose(1, 0, 2).reshape(128, -1)
        )

    W_comb = (W_toP + (W_U * W_V[None, :, :]).sum(-1)).astype(np.float32)
    WB = W_comb @ np.asarray(B_mat, np.float32)
    f8 = ml_dtypes.float8_e4m3fn
    wf8 = swz(np.asarray(W_F, np.float32)).astype(f8)   # [128, KD*P] fp8
    wbs = swz(WB)                                        # [128, KD*PR]
    wb = np.zeros((128, KD * 128), np.float32)
    for k in range(KD):
        wb[:, k * 128 : k * 128 + PR] = wbs[:, k * PR : (k + 1) * PR]
    wb = wb.astype(bf)
    wcomb = swz(W_comb).astype(bf)
    wlam_pad = np.zeros((P, 128), np.float32)
    wlam_pad[:, 0:PR] = np.asarray(W_lam, np.float32)
    wlam = swz(wlam_pad).astype(bf)
    cmat = np.zeros((128, P), np.float32)
    cmat[0:PR, :] = np.asarray(C_mat, np.float32)
    cmat = cmat.astype(bf)
    wfp = swz(np.asarray(W_fromP, np.float32)).astype(bf)  # [128, PT*D]
    in_maps = []
    for c in range(NCORES):
        b, half = c // 2, c % 2
        xT = np.asarray(x[b, half * TL : (half + 1) * TL, :], np.float32).T
        # [D, TL] -> [128, NCH*KD*CH] with (c, k, t) free order, partition-major
        xsf = np.ascontiguousarray(
            xT.reshape(KD, 128, NCH, CH).transpose(1, 2, 0, 3).reshape(128, -1)
        )
        xs = xsf.astype(bf)
        x8 = xsf.astype(f8)
        in_maps.append(
            {
                "xt": xs,
                "x8": x8,
                "wf8": wf8,
                "wb": wb,
                "wcomb": wcomb,
                "wlam": wlam,
                "cmat": cmat,
                "wfp": wfp,
            }
        )
    return in_maps


def kernel(**inputs) -> np.ndarray:
    inputs = {k: np.asarray(v) for k, v in inputs.items()}
    if "nc" not in _CACHE:
        _CACHE["nc"] = build_program()
    nc = _CACHE["nc"]
    in_maps = _prep_inputs(**inputs)
    trace = bool(int(os.environ.get("CEPTA_TRACE", "0")))
    res = bass_utils.run_bass_kernel_spmd(
        nc,
        in_maps,
        core_ids=list(range(NCORES)),
        trace=trace,
        trace_cores=[0] if trace else None,
    )
    _CACHE["last_result"] = res
    # host-side gather + cross-half scan-carry correction (bilinear in the
    # tiny cp [PR,CT] and sfin [PR] outputs; f32, ~0.03% of model FLOPs)
    mcw = (
        np.asarray(inputs["C_mat"], np.float32)
        @ np.asarray(inputs["W_fromP"], np.float32)
    )
    out = np.empty((B, T, D), np.float32)
    for b in range(B):
        even, odd = res.results[2 * b], res.results[2 * b + 1]
        h0 = even["h"].astype(np.float32)
        h1 = odd["h"].astype(np.float32)
        corr = (odd["cp"] * even["sfin"]).T @ mcw   # [CT, D]
        h1[0:CT] += corr
        out[b, 0:TL] = h0
        out[b, TL:T] = h1
    return out


# revision 49
# speedup vs baseline: 1.0933x; 1.0549x over previous
"""Trainium2 Bass kernel for nn_CeptaContextBlock (B=4, T=4096, D=1024, P=512, ALPHA=4, PR=64).

Math (after algebraic simplification of the reference):
    W_comb = W_toP + sum_a W_U[:,:,a] * W_V[:,a]          (host precompute)
    WB     = W_comb @ B_mat                               (host precompute)
    Fg   = sigmoid(x @ W_F)                               (B,T,P)
    lam  = sigmoid(Fg @ W_lam)                            (B,T,PR)
    u    = x @ WB          (== (x @ W_comb) @ B_mat)      (B,T,PR)
    s    = scan: s_i = lam_i * s_{i-1} + u_i along T      (B,T,PR)
    t_til= x @ W_comb + s @ C_mat                         (B,T,P)
    h    = t_til @ W_fromP                                (B,T,D)

Sharding: 8 cores; core c handles batch b=c//2, token half c%2 (2048 tokens).
The cross-half scan dependency is NOT exchanged on device (an AllGather costs
~34us doorbell-to-data). Instead each core outputs its final scan state
sfin=s[TL-1] (64 floats) and cp[r,t]=prod_{i<=t} lam[r,i] over its first CT
tokens; cp underflows to exactly 0 within ~200 tokens. The host applies the
bilinear correction h_odd[0:CT] += (cp_odd * sfin_even[:,None]).T @ (C@W_fromP)
during the gather step (f32, ~0.03% of the model FLOPs).

On-device schedule: chunk-sequential pipeline (A_c, T_c, H_c) x 4:
  A_c = Fg (chunk 0: k-outer 4-wide to match the DMA ramp; else k-inner),
        u, lam, chained scan
  T_c = 9-matmul PSUM groups (8x wcomb + 1x C@sloc, the close deferred one
        group so the scan/cast chain never stalls the PE) -> t_til (bf16)
  H_c = t_til @ W_fromP, per-dc copy (scalar/vector) + per-dc DMA
Every stationary is a full 128-column load (u/lam/C operands are padded on
the host; narrow LDWEIGHTS measured +95ns each). Input DMAs ride both HWDGE
queues as column slices sized to the DGE's slow first ~10us, with wcrit's
back half on the SWDGE (gpsimd) queue as a third lane. There are no
collectives and no cross-core traffic.
"""

import os
import sys

import numpy as np

for _p in ("/opt/trn_rl_repo", "/root/.axon_site/_ro/trn_rl_repo"):
    if os.path.isdir(_p) and _p not in sys.path:
        sys.path.append(_p)

import ml_dtypes

import concourse.bass as bass
import concourse.bacc as bacc
import concourse.mybir as mybir
import concourse.tile as tile
from concourse import bass_utils

B, T, D, P, ALPHA, PR = 4, 4096, 1024, 512, 4, 64
NCORES = 8
TL = T // 2          # tokens per core
KD = D // 128        # 8 d-chunks (contraction for the big matmuls)
PT = P // 128        # 4 p-tiles
CH = 512             # token chunk (free dim per matmul)
NCH = TL // CH       # 4 token chunks per core
CRIT = P + 128       # packed scan-critical weight: [W_F | WB | 0-pad] per k-chunk
CT = 256             # correction window (cumprod(lam) ~ 0 well before this)
F32 = mybir.dt.float32
BF16 = mybir.dt.bfloat16
F8 = mybir.dt.float8e4
DR = mybir.MatmulPerfMode.DoubleRow
SIG = mybir.ActivationFunctionType.Sigmoid
CPY = mybir.ActivationFunctionType.Copy
MUL = mybir.AluOpType.mult
ADD = mybir.AluOpType.add
BYP = mybir.AluOpType.bypass

_CACHE = {}


def build_program(ncores: int = NCORES):
    """Build the SPMD Tile program (same NEFF on all cores, no collectives)."""
    nc = bacc.Bacc(
        "TRN2", target_bir_lowering=False, debug=False, num_devices=ncores
    )

    # k-block-major DRAM layouts: every per-k DMA slice is one fully
    # contiguous run ([128, cols] block with adjacent partitions). The
    # narrow (64-wide) stationaries are zero-padded to 128 so every
    # LDWEIGHTS is a standard full-width load that pipelines behind the
    # in-flight matmul (narrow LDW measured +95ns serialization each).
    xt_d = nc.dram_tensor("xt", [128, NCH * KD * CH], BF16, kind="ExternalInput")
    x8_d = nc.dram_tensor("x8", [128, NCH * KD * CH], F8, kind="ExternalInput")
    wf8_d = nc.dram_tensor("wf8", [128, KD * P], F8, kind="ExternalInput")
    wb_d = nc.dram_tensor("wb", [128, KD * 128], BF16, kind="ExternalInput")
    wcomb_d = nc.dram_tensor("wcomb", [128, KD * P], BF16, kind="ExternalInput")
    wlam_d = nc.dram_tensor("wlam", [128, PT * 128], BF16, kind="ExternalInput")
    cmat_d = nc.dram_tensor("cmat", [128, P], BF16, kind="ExternalInput")
    wfp_d = nc.dram_tensor("wfp", [128, PT * D], BF16, kind="ExternalInput")
    h_d = nc.dram_tensor("h", [TL, D], BF16, kind="ExternalOutput")
    sfin_d = nc.dram_tensor("sfin", [PR, 1], F32, kind="ExternalOutput")
    cp_d = nc.dram_tensor("cp", [PR, CT], F32, kind="ExternalOutput")

    xt_vc = xt_d.rearrange("p (c q) -> p c q", c=NCH)
    x8_vc = x8_d.rearrange("p (c q) -> p c q", c=NCH)

    with tile.TileContext(nc) as tc:
        with (
            tc.tile_pool(name="wp", bufs=1) as wp,
            tc.tile_pool(name="xp", bufs=4) as xp,
            tc.tile_pool(name="fgp", bufs=2) as fgp,
            tc.tile_pool(name="ttp", bufs=2) as ttp,
            tc.tile_pool(name="sp", bufs=2) as sp,
            tc.tile_pool(name="big", bufs=1) as big,
            tc.tile_pool(name="hp", bufs=8) as hp,
            tc.tile_pool(name="ppa", bufs=2, space="PSUM") as ppa,
            tc.tile_pool(name="pps", bufs=1, space="PSUM") as pps,
            tc.tile_pool(name="ppt", bufs=3, space="PSUM") as ppt,
            tc.tile_pool(name="pph", bufs=2, space="PSUM") as pph,
        ):
            # ---- input DMAs. Two HWDGE queues (sync/scalar) carry the
            # critical stream as quarter-column slices (2.3-4.6KB/partition
            # runs stream at full rate; per-descriptor latency ~1us, so 4
            # slices/tensor balances start latency vs throughput). gpsimd
            # (SWDGE, ~60GB/s) hauls the late-deadline weights as bonus
            # bandwidth. ----
            wf8_sb = wp.tile([128, KD * P], F8, tag="wf8", name="wf8_sb")
            wb_sb = wp.tile([128, KD * 128], BF16, tag="wb", name="wb_sb")
            xt_tiles = [
                xp.tile([128, KD * CH], BF16, tag="xt", name=f"xt{c}")
                for c in range(NCH)
            ]
            x8_tiles = [
                xp.tile([128, KD * CH], F8, tag="x8", name=f"x8_{c}")
                for c in range(NCH)
            ]
            # ramp (sync/scalar HWDGE): fp8 W_F + fp8 x chunk 0 in quarter
            # slices — the Fg path needs only ~1MB before full speed
            for q in range(4):
                nc.sync.dma_start(
                    wf8_sb[:, q * 2 * P : (q + 1) * 2 * P],
                    wf8_d[:, q * 2 * P : (q + 1) * 2 * P],
                )
                nc.scalar.dma_start(
                    x8_tiles[0][:, q * 2 * CH : (q + 1) * 2 * CH],
                    x8_vc[:, 0, q * 2 * CH : (q + 1) * 2 * CH],
                )
            # behind the ramp, ordered by deadline (A-phases halved by
            # DoubleRow, so every deadline moved earlier): sync carries
            # wlam/wcomb/wfp + the later fp8 chunks; scalar carries the bf16
            # x + WB for the u path
            wlam_sb = wp.tile([128, PT * 128], BF16, tag="wlam", name="wlam_sb")
            nc.sync.dma_start(wlam_sb[:], wlam_d[:, :])
            nc.scalar.dma_start(xt_tiles[0][:], xt_vc[:, 0, :])
            wcomb_sb = wp.tile([128, KD * P], BF16, tag="wcomb", name="wcomb_sb")
            nc.sync.dma_start(wcomb_sb[:], wcomb_d[:, :])
            nc.scalar.dma_start(wb_sb[:], wb_d[:, :])
            nc.sync.dma_start(x8_tiles[1][:], x8_vc[:, 1, :])
            wfp_sb = wp.tile([128, PT * D], BF16, tag="wfp", name="wfp_sb")
            nc.sync.dma_start(wfp_sb[:], wfp_d[:, :])
            nc.scalar.dma_start(xt_tiles[1][:], xt_vc[:, 1, :])
            nc.sync.dma_start(x8_tiles[2][:], x8_vc[:, 2, :])
            nc.scalar.dma_start(xt_tiles[2][:], xt_vc[:, 2, :])
            nc.sync.dma_start(x8_tiles[3][:], x8_vc[:, 3, :])
            nc.scalar.dma_start(xt_tiles[3][:], xt_vc[:, 3, :])
            cmat_sb = wp.tile([128, P], BF16, tag="cmat", name="cmat_sb")
            nc.gpsimd.dma_start(cmat_sb[:], cmat_d[:, :])

            # (a PE pre-warm with junk matmuls was tried and measured
            # net-neutral: the ramp is DMA-bound, so the cold-clock matmuls
            # already hide inside DMA stalls)

            # ---- persistent activations ----
            s1_sb = big.tile([PR, TL], F32, tag="s1", name="s1")
            cp_sb = big.tile([PR, CT], F32, tag="cp", name="cp")
            # sloc holds bf16 s per chunk on partitions 0:64; partitions
            # 64:128 feed the zero rows of the padded C stationary and are
            # zeroed once (uninitialized SBUF could hold NaN, and NaN*0=NaN)
            sloc_sb = big.tile([128, TL], BF16, tag="sloc", name="sloc")
            nc.vector.memset(sloc_sb[64:128, :], 0.0)

            def phase_a(c):
                """Fg (fp8 DoubleRow), u, lam, chained scan for chunk c."""
                cs = slice(c * CH, (c + 1) * CH)
                xt_c = xt_tiles[c]
                # 3D views: [p, 2, f] pairs two k-chunks per DoubleRow matmul
                # (out = sum_i W[:,i].T @ X[:,i] — a 256-deep contraction)
                wfv = wf8_sb[:].rearrange("p (k q) -> p k q", k=KD)
                x8v = x8_tiles[c][:].rearrange("p (k q) -> p k q", k=KD)
                fg_c = [
                    fgp.tile([128, CH], BF16, tag=f"fg{m}", name=f"fg{c}_{m}")
                    for m in range(PT)
                ]
                pa = [
                    ppa.tile([128, CH], F32, tag="pa", name=f"pa{c}_{j}")
                    for j in range(2)
                ]
                if c == 0:
                    # pair-outer, 4-wide: all four p-tiles advance per k-pair,
                    # matching the DMA ramp. Banks 2/3 borrow the T-pool's
                    # slots (sequential lifetimes: sigmoids free them pre-T0)
                    pa = pa + [
                        ppt.tile([128, CH], F32, tag="pt", name=f"pa0x{j}")
                        for j in range(2)
                    ]
                    for q in range(KD // 2):
                        for m in range(PT):
                            nc.tensor.matmul(
                                pa[m][:],
                                wfv[:, 2 * q : 2 * q + 2, m * 128 : (m + 1) * 128],
                                x8v[:, 2 * q : 2 * q + 2, :],
                                start=(q == 0),
                                stop=(q == KD // 2 - 1),
                                perf_mode=DR,
                            )
                    for m in range(PT):
                        nc.scalar.activation(fg_c[m][:], pa[m][:], SIG)
                else:
                    # pair-inner, m-outer: each group's bank frees via its
                    # sigmoid while the next group runs (no convoy)
                    for m in range(PT):
                        pam = pa[m % 2] if m < 2 else ppa.tile(
                            [128, CH], F32, tag="pa", name=f"pa{c}b{m}"
                        )
                        for q in range(KD // 2):
                            nc.tensor.matmul(
                                pam[:],
                                wfv[:, 2 * q : 2 * q + 2, m * 128 : (m + 1) * 128],
                                x8v[:, 2 * q : 2 * q + 2, :],
                                start=(q == 0),
                                stop=(q == KD // 2 - 1),
                                perf_mode=DR,
                            )
                        nc.scalar.activation(fg_c[m][:], pam[:], SIG)
                # lam = sigmoid(Fg @ [W_lam | 0]); pl borrows a ppa slot (the
                # fg banks are sigmoided-free by the time lam runs)
                pl = ppa.tile([128, CH], F32, tag="pa", name=f"pl{c}")
                for m in range(PT):
                    nc.tensor.matmul(
                        pl[:],
                        wlam_sb[:, m * 128 : (m + 1) * 128],
                        fg_c[m][:],
                        start=(m == 0),
                        stop=(m == PT - 1),
                    )
                lam_c = sp.tile([PR, CH], F32, tag="lam", name=f"lam{c}")
                nc.scalar.activation(lam_c[:], pl[0:PR, :], SIG)
                # u = x @ [WB | 0] (bf16 x; the scan consumes u directly, so
                # it cannot ride the fp8 path)
                pu = pps.tile([128, CH], F32, tag="ps", name=f"pu{c}")
                for k in range(KD):
                    nc.tensor.matmul(
                        pu[:],
                        wb_sb[:, k * 128 : (k + 1) * 128],
                        xt_c[:, k * CH : (k + 1) * CH],
                        start=(k == 0),
                        stop=(k == KD - 1),
                    )
                # chained local scan; u consumed straight from PSUM
                init = 0.0 if c == 0 else s1_sb[:, c * CH - 1 : c * CH]
                nc.vector.tensor_tensor_scan(
                    s1_sb[:, cs], lam_c[:], pu[0:PR, :], init, op0=MUL, op1=ADD
                )
                if c == 0:
                    nc.vector.tensor_tensor_scan(
                        cp_sb[:], lam_c[:, 0:CT], lam_c[:, 0:CT], 1.0,
                        op0=MUL, op1=BYP,
                    )
                    nc.gpsimd.dma_start(cp_d[:, :], cp_sb[:])
                nc.vector.tensor_copy(sloc_sb[0:PR, cs], s1_sb[:, cs])
                if c == NCH - 1:
                    nc.gpsimd.dma_start(sfin_d[:, :], s1_sb[:, TL - 1 : TL])

            def phase_t(c):
                """t_til = x@W_comb + s@C as one 9-mm PSUM group per p-tile.
                The C stationary is row-padded to 128 (rows 64:128 zero), so
                the s matmul is a standard full-width load too. Each group's
                closing s@C matmul is deferred behind the NEXT group's wcomb
                run, giving the scan->sloc-cast chain ~3.5us of PE cover."""
                cs = slice(c * CH, (c + 1) * CH)
                xt_c = xt_tiles[c]
                ttil_c = [
                    ttp.tile([128, CH], BF16, tag=f"tt{m}", name=f"ttil{c}_{m}")
                    for m in range(PT)
                ]
                pt_ = [None] * PT

                def sc_close(m):
                    nc.tensor.matmul(
                        pt_[m][:],
                        cmat_sb[:, m * 128 : (m + 1) * 128],
                        sloc_sb[:, cs],
                        start=False,
                        stop=True,
                    )
                    nc.vector.tensor_copy(ttil_c[m][:], pt_[m][:])

                for m in range(PT):
                    pt_[m] = ppt.tile([128, CH], F32, tag="pt", name=f"pt{c}_{m}")
                    for k in range(KD):
                        nc.tensor.matmul(
                            pt_[m][:],
                            wcomb_sb[:, k * P + m * 128 : k * P + (m + 1) * 128],
                            xt_c[:, k * CH : (k + 1) * CH],
                            start=(k == 0),
                            stop=False,
                        )
                    if m >= 1:
                        sc_close(m - 1)
                sc_close(PT - 1)
                return ttil_c

            def phase_h(c, ttil_c):
                """h = t_til @ W_fromP, streamed out per 128-token tile."""
                for tt in range(CH // 128):
                    ts_ = slice(tt * 128, (tt + 1) * 128)
                    rs = slice((c * 4 + tt) * 128, (c * 4 + tt + 1) * 128)
                    h_t = hp.tile([128, D], BF16, tag="hs", name=f"h{c}_{tt}")
                    for dc in range(2):
                        dcs = slice(dc * CH, (dc + 1) * CH)
                        ph = pph.tile([128, CH], F32, tag="ph", name=f"ph{c}_{tt}_{dc}")
                        for m in range(PT):
                            nc.tensor.matmul(
                                ph[:],
                                ttil_c[m][:, ts_],
                                wfp_sb[:, m * D + dc * CH : m * D + dc * CH + CH],
                                start=(m == 0),
                                stop=(m == PT - 1),
                            )
                        last = c == NCH - 1 and tt == 3 and dc == 1
                        if last:
                            # final copy split across both engines; its two
                            # DMA halves ride both queues — shortest tail
                            nc.scalar.activation(
                                h_t[:, CH : CH + 256], ph[:, 0:256], CPY
                            )
                            nc.vector.tensor_copy(
                                h_t[:, CH + 256 : D], ph[:, 256:CH]
                            )
                            nc.sync.dma_start(
                                h_d[rs, CH : CH + 256], h_t[:, CH : CH + 256]
                            )
                            nc.scalar.dma_start(
                                h_d[rs, CH + 256 : D], h_t[:, CH + 256 : D]
                            )
                        else:
                            if dc == 0:
                                nc.scalar.activation(h_t[:, dcs], ph[:], CPY)
                            else:
                                nc.vector.tensor_copy(h_t[:, dcs], ph[:])
                            # per-dc DMA right behind its copy: fine-grained
                            # waits keep the output stream flowing (a per-tile
                            # DMA gets a coarsened vector-clock wait and
                            # bunches up)
                            nc.sync.dma_start(h_d[rs, dcs], h_t[:, dcs])

            # ---- chunk-sequential pipeline: chunk 0's full A/T/H covers
            # the input-DMA window, pushing xt1's deadline to ~35us ----
            for c in range(NCH):
                phase_a(c)
                tt_c = phase_t(c)
                phase_h(c, tt_c)

    nc.compile()
    return nc


def _prep_inputs(x, W_toP, W_U, W_F, W_V, W_lam, B_mat, C_mat, W_fromP):
    """Host-side sharding prep: weight folds, bf16 cast, per-core x transpose."""
    bf = ml_dtypes.bfloat16
    def swz(w):
        # [K*128, q] -> partition-major [128, K*q]
        kq = w.shape[0] // 128
        return np.ascontiguousarray(
            w.reshape(kq, 128, w.shape[1]).transpose(1, 0, 2).reshape(128, -1)
        )

    W_comb = (W_toP + (W_U * W_V[None, :, :]).sum(-1)).astype(np.float32)
    WB = W_comb @ np.asarray(B_mat, np.float32)
    f8 = ml_dtypes.float8_e4m3fn
    wf8 = swz(np.asarray(W_F, np.float32)).astype(f8)   # [128, KD*P] fp8
    wbs = swz(WB)                                        # [128, KD*PR]
    wb = np.zeros((128, KD * 128), np.float32)
    for k in range(KD):
        wb[:, k * 128 : k * 128 + PR] = wbs[:, k * PR : (k + 1) * PR]
    wb = wb.astype(bf)
    wcomb = swz(W_comb).astype(bf)
    wlam_pad = np.zeros((P, 128), np.float32)
    wlam_pad[:, 0:PR] = np.asarray(W_lam, np.float32)
    wlam = swz(wlam_pad).astype(bf)
    cmat = np.zeros((128, P), np.float32)
    cmat[0:PR, :] = np.asarray(C_mat, np.float32)
    cmat = cmat.astype(bf)
    wfp = swz(np.asarray(W_fromP, np.float32)).astype(bf)  # [128, PT*D]
    in_maps = []
    for c in range(NCORES):
        b, half = c // 2, c % 2
        xT = np.asarray(x[b, half * TL : (half + 1) * TL, :], np.float32).T
        # [D, TL] -> [128, NCH*KD*CH] with (c, k, t) free order, partition-major
        xsf = np.ascontiguousarray(
            xT.reshape(KD, 128, NCH, CH).transpose(1, 2, 0, 3).reshape(128, -1)
        )
        xs = xsf.astype(bf)
        x8 = xsf.astype(f8)
        in_maps.append(
            {
                "xt": xs,
                "x8": x8,
                "wf8": wf8,
                "wb": wb,
                "wcomb": wcomb,
                "wlam": wlam,
                "cmat": cmat,
                "wfp": wfp,
            }
        )
    return in_maps


def kernel(**inputs) -> np.ndarray:
    inputs = {k: np.asarray(v) for k, v in inputs.items()}
    if "nc" not in _CACHE:
        _CACHE["nc"] = build_program()
    nc = _CACHE["nc"]
    in_maps = _prep_inputs(**inputs)
    trace = bool(int(os.environ.get("CEPTA_TRACE", "0")))
    res = bass_utils.run_bass_kernel_spmd(
        nc,
        in_maps,
        core_ids=list(range(NCORES)),
        trace=trace,
        trace_cores=[0] if trace else None,
    )
    _CACHE["last_result"] = res
    # host-side gather + cross-half scan-carry correction (bilinear in the
    # tiny cp [PR,CT] and sfin [PR] outputs; f32, ~0.03% of model FLOPs)
    mcw = (
        np.asarray(inputs["C_mat"], np.float32)
        @ np.asarray(inputs["W_fromP"], np.float32)
    )
    out = np.empty((B, T, D), np.float32)
    for b in range(B):
        even, odd = res.results[2 * b], res.results[2 * b + 1]
        h0 = even["h"].astype(np.float32)
        h1 = odd["h"].astype(np.float32)
        corr = (odd["cp"] * even["sfin"]).T @ mcw   # [CT, D]
        h1[0:CT] += corr
        out[b, 0:TL] = h0
        out[b, TL:T] = h1
    return out


# revision 50
# speedup vs baseline: 1.0940x; 1.0007x over previous
"""Trainium2 Bass kernel for nn_CeptaContextBlock (B=4, T=4096, D=1024, P=512, ALPHA=4, PR=64).

Math (after algebraic simplification of the reference):
    W_comb = W_toP + sum_a W_U[:,:,a] * W_V[:,a]          (host precompute)
    WB     = W_comb @ B_mat                               (host precompute)
    Fg   = sigmoid(x @ W_F)                               (B,T,P)
    lam  = sigmoid(Fg @ W_lam)                            (B,T,PR)
    u    = x @ WB          (== (x @ W_comb) @ B_mat)      (B,T,PR)
    s    = scan: s_i = lam_i * s_{i-1} + u_i along T      (B,T,PR)
    t_til= x @ W_comb + s @ C_mat                         (B,T,P)
    h    = t_til @ W_fromP                                (B,T,D)

Sharding: 8 cores; core c handles batch b=c//2, token half c%2 (2048 tokens).
The cross-half scan dependency is NOT exchanged on device (an AllGather costs
~34us doorbell-to-data). Instead each core outputs its final scan state
sfin=s[TL-1] (64 floats) and cp[r,t]=prod_{i<=t} lam[r,i] over its first CT
tokens; cp underflows to exactly 0 within ~200 tokens. The host applies the
bilinear correction h_odd[0:CT] += (cp_odd * sfin_even[:,None]).T @ (C@W_fromP)
during the gather step (f32, ~0.03% of the model FLOPs).

On-device schedule: chunk-sequential pipeline (A_c, T_c, H_c) x 4:
  A_c = Fg (chunk 0: k-outer 4-wide to match the DMA ramp; else k-inner),
        u, lam, chained scan
  T_c = 9-matmul PSUM groups (8x wcomb + 1x C@sloc, the close deferred one
        group so the scan/cast chain never stalls the PE) -> t_til (bf16)
  H_c = t_til @ W_fromP, per-dc copy (scalar/vector) + per-dc DMA
Every stationary is a full 128-column load (u/lam/C operands are padded on
the host; narrow LDWEIGHTS measured +95ns each). Input DMAs ride both HWDGE
queues as column slices sized to the DGE's slow first ~10us, with wcrit's
back half on the SWDGE (gpsimd) queue as a third lane. There are no
collectives and no cross-core traffic.
"""

import os
import sys

import numpy as np

for _p in ("/opt/trn_rl_repo", "/root/.axon_site/_ro/trn_rl_repo"):
    if os.path.isdir(_p) and _p not in sys.path:
        sys.path.append(_p)

import ml_dtypes

import concourse.bass as bass
import concourse.bacc as bacc
import concourse.mybir as mybir
import concourse.tile as tile
from concourse import bass_utils

B, T, D, P, ALPHA, PR = 4, 4096, 1024, 512, 4, 64
NCORES = 8
TL = T // 2          # tokens per core
KD = D // 128        # 8 d-chunks (contraction for the big matmuls)
PT = P // 128        # 4 p-tiles
CH = 512             # token chunk (free dim per matmul)
NCH = TL // CH       # 4 token chunks per core
CRIT = P + 128       # packed scan-critical weight: [W_F | WB | 0-pad] per k-chunk
CT = 256             # correction window (cumprod(lam) ~ 0 well before this)
F32 = mybir.dt.float32
BF16 = mybir.dt.bfloat16
F8 = mybir.dt.float8e4
DR = mybir.MatmulPerfMode.DoubleRow
SIG = mybir.ActivationFunctionType.Sigmoid
CPY = mybir.ActivationFunctionType.Copy
MUL = mybir.AluOpType.mult
ADD = mybir.AluOpType.add
BYP = mybir.AluOpType.bypass

_CACHE = {}


def build_program(ncores: int = NCORES):
    """Build the SPMD Tile program (same NEFF on all cores, no collectives)."""
    nc = bacc.Bacc(
        "TRN2", target_bir_lowering=False, debug=False, num_devices=ncores
    )

    # k-block-major DRAM layouts: every per-k DMA slice is one fully
    # contiguous run ([128, cols] block with adjacent partitions). The
    # narrow (64-wide) stationaries are zero-padded to 128 so every
    # LDWEIGHTS is a standard full-width load that pipelines behind the
    # in-flight matmul (narrow LDW measured +95ns serialization each).
    xt_d = nc.dram_tensor("xt", [128, NCH * KD * CH], BF16, kind="ExternalInput")
    x8_d = nc.dram_tensor("x8", [128, NCH * KD * CH], F8, kind="ExternalInput")
    wf8_d = nc.dram_tensor("wf8", [128, KD * P], F8, kind="ExternalInput")
    wb_d = nc.dram_tensor("wb", [128, KD * 128], BF16, kind="ExternalInput")
    wcomb_d = nc.dram_tensor("wcomb", [128, KD * P], BF16, kind="ExternalInput")
    wlam_d = nc.dram_tensor("wlam", [128, PT * 128], BF16, kind="ExternalInput")
    cmat_d = nc.dram_tensor("cmat", [128, P], BF16, kind="ExternalInput")
    wfp_d = nc.dram_tensor("wfp", [128, PT * D], BF16, kind="ExternalInput")
    h_d = nc.dram_tensor("h", [TL, D], BF16, kind="ExternalOutput")
    sfin_d = nc.dram_tensor("sfin", [PR, 1], F32, kind="ExternalOutput")
    cp_d = nc.dram_tensor("cp", [PR, CT], F32, kind="ExternalOutput")

    xt_vc = xt_d.rearrange("p (c q) -> p c q", c=NCH)
    x8_vc = x8_d.rearrange("p (c q) -> p c q", c=NCH)

    with tile.TileContext(nc) as tc:
        with (
            tc.tile_pool(name="wp", bufs=1) as wp,
            tc.tile_pool(name="xp", bufs=4) as xp,
            tc.tile_pool(name="fgp", bufs=2) as fgp,
            tc.tile_pool(name="ttp", bufs=2) as ttp,
            tc.tile_pool(name="sp", bufs=2) as sp,
            tc.tile_pool(name="big", bufs=1) as big,
            tc.tile_pool(name="hp", bufs=8) as hp,
            tc.tile_pool(name="ppa", bufs=2, space="PSUM") as ppa,
            tc.tile_pool(name="pps", bufs=1, space="PSUM") as pps,
            tc.tile_pool(name="ppt", bufs=3, space="PSUM") as ppt,
            tc.tile_pool(name="pph", bufs=2, space="PSUM") as pph,
        ):
            # ---- input DMAs. Two HWDGE queues (sync/scalar) carry the
            # critical stream as quarter-column slices (2.3-4.6KB/partition
            # runs stream at full rate; per-descriptor latency ~1us, so 4
            # slices/tensor balances start latency vs throughput). gpsimd
            # (SWDGE, ~60GB/s) hauls the late-deadline weights as bonus
            # bandwidth. ----
            wf8_sb = wp.tile([128, KD * P], F8, tag="wf8", name="wf8_sb")
            wb_sb = wp.tile([128, KD * 128], BF16, tag="wb", name="wb_sb")
            xt_tiles = [
                xp.tile([128, KD * CH], BF16, tag="xt", name=f"xt{c}")
                for c in range(NCH)
            ]
            x8_tiles = [
                xp.tile([128, KD * CH], F8, tag="x8", name=f"x8_{c}")
                for c in range(NCH)
            ]
            # ramp (sync/scalar HWDGE): fp8 W_F + fp8 x chunk 0 in quarter
            # slices — the Fg path needs only ~1MB before full speed
            for q in range(4):
                nc.sync.dma_start(
                    wf8_sb[:, q * 2 * P : (q + 1) * 2 * P],
                    wf8_d[:, q * 2 * P : (q + 1) * 2 * P],
                )
                nc.scalar.dma_start(
                    x8_tiles[0][:, q * 2 * CH : (q + 1) * 2 * CH],
                    x8_vc[:, 0, q * 2 * CH : (q + 1) * 2 * CH],
                )
            # behind the ramp, ordered by deadline (A-phases halved by
            # DoubleRow, so every deadline moved earlier): sync carries
            # wlam/wcomb/wfp + the later fp8 chunks; scalar carries the bf16
            # x + WB for the u path
            # wlam rides SWDGE (lands ~9.5us, deadline ~17); wcomb/wfp lead
            # the sync queue behind the wf8 ramp; bf16 x + WB on scalar
            wlam_sb = wp.tile([128, PT * 128], BF16, tag="wlam", name="wlam_sb")
            nc.gpsimd.dma_start(wlam_sb[:], wlam_d[:, :])
            nc.scalar.dma_start(xt_tiles[0][:], xt_vc[:, 0, :])
            wcomb_sb = wp.tile([128, KD * P], BF16, tag="wcomb", name="wcomb_sb")
            nc.sync.dma_start(wcomb_sb[:], wcomb_d[:, :])
            nc.scalar.dma_start(wb_sb[:], wb_d[:, :])
            wfp_sb = wp.tile([128, PT * D], BF16, tag="wfp", name="wfp_sb")
            nc.sync.dma_start(wfp_sb[:], wfp_d[:, :])
            nc.sync.dma_start(x8_tiles[1][:], x8_vc[:, 1, :])
            nc.scalar.dma_start(xt_tiles[1][:], xt_vc[:, 1, :])
            nc.sync.dma_start(x8_tiles[2][:], x8_vc[:, 2, :])
            nc.scalar.dma_start(xt_tiles[2][:], xt_vc[:, 2, :])
            nc.sync.dma_start(x8_tiles[3][:], x8_vc[:, 3, :])
            nc.scalar.dma_start(xt_tiles[3][:], xt_vc[:, 3, :])
            cmat_sb = wp.tile([128, P], BF16, tag="cmat", name="cmat_sb")
            nc.gpsimd.dma_start(cmat_sb[:], cmat_d[:, :])

            # (a PE pre-warm with junk matmuls was tried and measured
            # net-neutral: the ramp is DMA-bound, so the cold-clock matmuls
            # already hide inside DMA stalls)

            # ---- persistent activations ----
            s1_sb = big.tile([PR, TL], F32, tag="s1", name="s1")
            cp_sb = big.tile([PR, CT], F32, tag="cp", name="cp")
            # sloc holds bf16 s per chunk on partitions 0:64; partitions
            # 64:128 feed the zero rows of the padded C stationary and are
            # zeroed once (uninitialized SBUF could hold NaN, and NaN*0=NaN)
            sloc_sb = big.tile([128, TL], BF16, tag="sloc", name="sloc")
            nc.vector.memset(sloc_sb[64:128, :], 0.0)

            def phase_a(c):
                """Fg (fp8 DoubleRow), u, lam, chained scan for chunk c."""
                cs = slice(c * CH, (c + 1) * CH)
                xt_c = xt_tiles[c]
                # 3D views: [p, 2, f] pairs two k-chunks per DoubleRow matmul
                # (out = sum_i W[:,i].T @ X[:,i] — a 256-deep contraction)
                wfv = wf8_sb[:].rearrange("p (k q) -> p k q", k=KD)
                x8v = x8_tiles[c][:].rearrange("p (k q) -> p k q", k=KD)
                fg_c = [
                    fgp.tile([128, CH], BF16, tag=f"fg{m}", name=f"fg{c}_{m}")
                    for m in range(PT)
                ]
                pa = [
                    ppa.tile([128, CH], F32, tag="pa", name=f"pa{c}_{j}")
                    for j in range(2)
                ]
                if c == 0:
                    # pair-outer, 4-wide: all four p-tiles advance per k-pair,
                    # matching the DMA ramp. Banks 2/3 borrow the T-pool's
                    # slots (sequential lifetimes: sigmoids free them pre-T0)
                    pa = pa + [
                        ppt.tile([128, CH], F32, tag="pt", name=f"pa0x{j}")
                        for j in range(2)
                    ]
                    for q in range(KD // 2):
                        for m in range(PT):
                            nc.tensor.matmul(
                                pa[m][:],
                                wfv[:, 2 * q : 2 * q + 2, m * 128 : (m + 1) * 128],
                                x8v[:, 2 * q : 2 * q + 2, :],
                                start=(q == 0),
                                stop=(q == KD // 2 - 1),
                                perf_mode=DR,
                            )
                    for m in range(PT):
                        nc.scalar.activation(fg_c[m][:], pa[m][:], SIG)
                else:
                    # pair-inner, m-outer: each group's bank frees via its
                    # sigmoid while the next group runs (no convoy)
                    for m in range(PT):
                        pam = pa[m % 2] if m < 2 else ppa.tile(
                            [128, CH], F32, tag="pa", name=f"pa{c}b{m}"
                        )
                        for q in range(KD // 2):
                            nc.tensor.matmul(
                                pam[:],
                                wfv[:, 2 * q : 2 * q + 2, m * 128 : (m + 1) * 128],
                                x8v[:, 2 * q : 2 * q + 2, :],
                                start=(q == 0),
                                stop=(q == KD // 2 - 1),
                                perf_mode=DR,
                            )
                        nc.scalar.activation(fg_c[m][:], pam[:], SIG)
                # lam = sigmoid(Fg @ [W_lam | 0]); pl borrows a ppa slot (the
                # fg banks are sigmoided-free by the time lam runs)
                pl = ppa.tile([128, CH], F32, tag="pa", name=f"pl{c}")
                for m in range(PT):
                    nc.tensor.matmul(
                        pl[:],
                        wlam_sb[:, m * 128 : (m + 1) * 128],
                        fg_c[m][:],
                        start=(m == 0),
                        stop=(m == PT - 1),
                    )
                lam_c = sp.tile([PR, CH], F32, tag="lam", name=f"lam{c}")
                nc.scalar.activation(lam_c[:], pl[0:PR, :], SIG)
                # u = x @ [WB | 0] (bf16 x; the scan consumes u directly, so
                # it cannot ride the fp8 path)
                pu = pps.tile([128, CH], F32, tag="ps", name=f"pu{c}")
                for k in range(KD):
                    nc.tensor.matmul(
                        pu[:],
                        wb_sb[:, k * 128 : (k + 1) * 128],
                        xt_c[:, k * CH : (k + 1) * CH],
                        start=(k == 0),
                        stop=(k == KD - 1),
                    )
                # chained local scan; u consumed straight from PSUM
                init = 0.0 if c == 0 else s1_sb[:, c * CH - 1 : c * CH]
                nc.vector.tensor_tensor_scan(
                    s1_sb[:, cs], lam_c[:], pu[0:PR, :], init, op0=MUL, op1=ADD
                )
                if c == 0:
                    nc.vector.tensor_tensor_scan(
                        cp_sb[:], lam_c[:, 0:CT], lam_c[:, 0:CT], 1.0,
                        op0=MUL, op1=BYP,
                    )
                    nc.gpsimd.dma_start(cp_d[:, :], cp_sb[:])
                nc.vector.tensor_copy(sloc_sb[0:PR, cs], s1_sb[:, cs])
                if c == NCH - 1:
                    nc.gpsimd.dma_start(sfin_d[:, :], s1_sb[:, TL - 1 : TL])

            def phase_t(c):
                """t_til = x@W_comb + s@C as one 9-mm PSUM group per p-tile.
                The C stationary is row-padded to 128 (rows 64:128 zero), so
                the s matmul is a standard full-width load too. Each group's
                closing s@C matmul is deferred behind the NEXT group's wcomb
                run, giving the scan->sloc-cast chain ~3.5us of PE cover."""
                cs = slice(c * CH, (c + 1) * CH)
                xt_c = xt_tiles[c]
                ttil_c = [
                    ttp.tile([128, CH], BF16, tag=f"tt{m}", name=f"ttil{c}_{m}")
                    for m in range(PT)
                ]
                pt_ = [None] * PT

                def sc_close(m):
                    nc.tensor.matmul(
                        pt_[m][:],
                        cmat_sb[:, m * 128 : (m + 1) * 128],
                        sloc_sb[:, cs],
                        start=False,
                        stop=True,
                    )
                    nc.vector.tensor_copy(ttil_c[m][:], pt_[m][:])

                for m in range(PT):
                    pt_[m] = ppt.tile([128, CH], F32, tag="pt", name=f"pt{c}_{m}")
                    for k in range(KD):
                        nc.tensor.matmul(
                            pt_[m][:],
                            wcomb_sb[:, k * P + m * 128 : k * P + (m + 1) * 128],
                            xt_c[:, k * CH : (k + 1) * CH],
                            start=(k == 0),
                            stop=False,
                        )
                    if m >= 1:
                        sc_close(m - 1)
                sc_close(PT - 1)
                return ttil_c

            def phase_h(c, ttil_c):
                """h = t_til @ W_fromP, streamed out per 128-token tile."""
                for tt in range(CH // 128):
                    ts_ = slice(tt * 128, (tt + 1) * 128)
                    rs = slice((c * 4 + tt) * 128, (c * 4 + tt + 1) * 128)
                    h_t = hp.tile([128, D], BF16, tag="hs", name=f"h{c}_{tt}")
                    for dc in range(2):
                        dcs = slice(dc * CH, (dc + 1) * CH)
                        ph = pph.tile([128, CH], F32, tag="ph", name=f"ph{c}_{tt}_{dc}")
                        for m in range(PT):
                            nc.tensor.matmul(
                                ph[:],
                                ttil_c[m][:, ts_],
                                wfp_sb[:, m * D + dc * CH : m * D + dc * CH + CH],
                                start=(m == 0),
                                stop=(m == PT - 1),
                            )
                        last = c == NCH - 1 and tt == 3 and dc == 1
                        if last:
                            # final copy split across both engines; its two
                            # DMA halves ride both queues — shortest tail
                            nc.scalar.activation(
                                h_t[:, CH : CH + 256], ph[:, 0:256], CPY
                            )
                            nc.vector.tensor_copy(
                                h_t[:, CH + 256 : D], ph[:, 256:CH]
                            )
                            nc.sync.dma_start(
                                h_d[rs, CH : CH + 256], h_t[:, CH : CH + 256]
                            )
                            nc.scalar.dma_start(
                                h_d[rs, CH + 256 : D], h_t[:, CH + 256 : D]
                            )
                        else:
                            if dc == 0:
                                nc.scalar.activation(h_t[:, dcs], ph[:], CPY)
                            else:
                                nc.vector.tensor_copy(h_t[:, dcs], ph[:])
                            # per-dc DMA right behind its copy: fine-grained
                            # waits keep the output stream flowing (a per-tile
                            # DMA gets a coarsened vector-clock wait and
                            # bunches up)
                            nc.sync.dma_start(h_d[rs, dcs], h_t[:, dcs])

            # ---- chunk-sequential pipeline: chunk 0's full A/T/H covers
            # the input-DMA window, pushing xt1's deadline to ~35us ----
            for c in range(NCH):
                phase_a(c)
                tt_c = phase_t(c)
                phase_h(c, tt_c)

    nc.compile()
    return nc


def _prep_inputs(x, W_toP, W_U, W_F, W_V, W_lam, B_mat, C_mat, W_fromP):
    """Host-side sharding prep: weight folds, bf16 cast, per-core x transpose."""
    bf = ml_dtypes.bfloat16
    def swz(w):
        # [K*128, q] -> partition-major [128, K*q]
        kq = w.shape[0] // 128
        return np.ascontiguousarray(
            w.reshape(kq, 128, w.shape[1]).transpose(1, 0, 2).reshape(128, -1)
        )

    W_comb = (W_toP + (W_U * W_V[None, :, :]).sum(-1)).astype(np.float32)
    WB = W_comb @ np.asarray(B_mat, np.float32)
    f8 = ml_dtypes.float8_e4m3fn
    wf8 = swz(np.asarray(W_F, np.float32)).astype(f8)   # [128, KD*P] fp8
    wbs = swz(WB)                                        # [128, KD*PR]
    wb = np.zeros((128, KD * 128), np.float32)
    for k in range(KD):
        wb[:, k * 128 : k * 128 + PR] = wbs[:, k * PR : (k + 1) * PR]
    wb = wb.astype(bf)
    wcomb = swz(W_comb).astype(bf)
    wlam_pad = np.zeros((P, 128), np.float32)
    wlam_pad[:, 0:PR] = np.asarray(W_lam, np.float32)
    wlam = swz(wlam_pad).astype(bf)
    cmat = np.zeros((128, P), np.float32)
    cmat[0:PR, :] = np.asarray(C_mat, np.float32)
    cmat = cmat.astype(bf)
    wfp = swz(np.asarray(W_fromP, np.float32)).astype(bf)  # [128, PT*D]
    in_maps = []
    for c in range(NCORES):
        b, half = c // 2, c % 2
        xT = np.asarray(x[b, half * TL : (half + 1) * TL, :], np.float32).T
        # [D, TL] -> [128, NCH*KD*CH] with (c, k, t) free order, partition-major
        xsf = np.ascontiguousarray(
            xT.reshape(KD, 128, NCH, CH).transpose(1, 2, 0, 3).reshape(128, -1)
        )
        xs = xsf.astype(bf)
        x8 = xsf.astype(f8)
        in_maps.append(
            {
                "xt": xs,
                "x8": x8,
                "wf8": wf8,
                "wb": wb,
                "wcomb": wcomb,
                "wlam": wlam,
                "cmat": cmat,
                "wfp": wfp,
            }
        )
    return in_maps


def kernel(**inputs) -> np.ndarray:
    inputs = {k: np.asarray(v) for k, v in inputs.items()}
    if "nc" not in _CACHE:
        _CACHE["nc"] = build_program()
    nc = _CACHE["nc"]
    in_maps = _prep_inputs(**inputs)
    trace = bool(int(os.environ.get("CEPTA_TRACE", "0")))
    res = bass_utils.run_bass_kernel_spmd(
        nc,
        in_maps,
        core_ids=list(range(NCORES)),
        trace=trace,
        trace_cores=[0] if trace else None,
    )
    _CACHE["last_result"] = res
    # host-side gather + cross-half scan-carry correction (bilinear in the
    # tiny cp [PR,CT] and sfin [PR] outputs; f32, ~0.03% of model FLOPs)
    mcw = (
        np.asarray(inputs["C_mat"], np.float32)
        @ np.asarray(inputs["W_fromP"], np.float32)
    )
    out = np.empty((B, T, D), np.float32)
    for b in range(B):
        even, odd = res.results[2 * b], res.results[2 * b + 1]
        h0 = even["h"].astype(np.float32)
        h1 = odd["h"].astype(np.float32)
        corr = (odd["cp"] * even["sfin"]).T @ mcw   # [CT, D]
        h1[0:CT] += corr
        out[b, 0:TL] = h0
        out[b, TL:T] = h1
    return out
